# revision 7
# baseline (speedup 1.0000x reference)
"""AMICO ADMM solver on 8 Trainium2 NeuronCores.

Problem: X = argmin ||Y^T - A x||^2 + lam*||x||_1 s.t. x >= 0, solved with
max_iter ADMM steps (rho=1, lam=0.1) exactly as in the reference scan.

Algebraic reduction (tracking only v = x + u):
    v_1 = G                      with G  = Minv @ A^T @ Y^T
    for i = 2..N:
        w   = |v - t|            (t = lam/rho)
        S   = min(v, t) + Gb     (Gb = G - t * Minv @ 1)
        v'  = Minv @ w + S
    output x_N = Minv @ w_{N-1} + Gb

since z = relu(v - t), u' = v - z = min(v, t), and z - u' = |v - t| - t.
The constant -t*Minv@1 and the A^T Y^T term are folded into a single
"augmented" matmul: Gb = Ht_aug^T @ Yt_aug where Ht_aug carries A@Minv plus a
bias row (-t * rowsum(Minv)) and Yt_aug carries Y^T plus a row of ones.

Sharding: data-parallel over voxels (B=4096 -> 512 per core); A-derived
matrices (Minv, Ht_aug) replicated; no cross-core communication.

Implementation notes (measured on silicon):
 - All matmul operands are fp16 (11-bit mantissa; fp32/fp16 mixing is
   rejected by the compiler, bf16 weights lose too much accuracy).
   End-to-end error vs the float32 cho_solve reference: ~5e-3.
 - Output chunks 0,1 use a DVE V-op (v = psum + S); chunks 2,3 instead
   accumulate I @ S_comb into the PSUM group via an identity matmul, so v
   materializes directly in PSUM and the Abs activation reads it from
   there, shortening the cross-iteration chain and balancing PE vs DVE.
 - PSUM tiles are allocated as bank-pairs [128, 1024] so the S_comb op for
   chunks 2,3 runs as a single wide DVE instruction (amortizes the ~150
   cycle DVE instruction overhead).
"""

import numpy as np

B_VOX = 4096
M_MEAS = 256
K_ATOMS = 512
P = 128
N_CORES = 8
BS = B_VOX // N_CORES  # 512 voxels per core
KB = K_ATOMS // P  # 4 chunks of the contraction/output dim
LAM = 0.1
RHO = 1.0
THR = LAM / RHO

_NC_CACHE = {}


def _build(niter):
    import concourse.mybir as mybir
    import concourse.tile as tile
    from concourse import bacc

    f32 = mybir.dt.float32
    f16 = mybir.dt.float16
    Alu = mybir.AluOpType
    Act = mybir.ActivationFunctionType

    nc = bacc.Bacc(None, target_bir_lowering=False)
    ht = nc.declare_dram_parameter("Ht", [3 * P, K_ATOMS], f16, isOutput=False)
    yt = nc.declare_dram_parameter("Yt", [3 * P, BS], f16, isOutput=False)
    mi = nc.declare_dram_parameter("Mi", [K_ATOMS, K_ATOMS], f16, isOutput=False)
    rs = nc.declare_dram_parameter("rs", [P, KB], f32, isOutput=False)
    ident = nc.declare_dram_parameter("Id", [P, P], f16, isOutput=False)
    out = nc.declare_dram_parameter("out", [K_ATOMS, BS], f32, isOutput=True)

    with tile.TileContext(nc) as tc:
        with (
            tc.tile_pool(name="const", bufs=1) as cpool,
            tc.tile_pool(name="v", bufs=8) as vpool,
            tc.tile_pool(name="w", bufs=12) as wpool,
            tc.tile_pool(name="s", bufs=6) as spool,
            tc.tile_pool(name="o", bufs=2) as opool,
            tc.tile_pool(name="psum", bufs=4, space="PSUM") as ppool,
        ):
            nb = cpool.tile([P, 1], f32)
            nc.vector.memset(nb[:], -THR)
            # split each big load into halves -> parallel DMA queues
            ht_sb = cpool.tile([P, 3, K_ATOMS], f16)
            htr = ht.rearrange("(kb p) a -> p kb a", p=P)
            nc.sync.dma_start(ht_sb[:, 0:2, :], htr[:, 0:2, :])
            nc.sync.dma_start(ht_sb[:, 2:3, :], htr[:, 2:3, :])
            yt_sb = cpool.tile([P, 3, BS], f16)
            ytr = yt.rearrange("(kb p) b -> p kb b", p=P)
            nc.sync.dma_start(yt_sb[:, 0:2, :], ytr[:, 0:2, :])
            nc.sync.dma_start(yt_sb[:, 2:3, :], ytr[:, 2:3, :])
            mi_sb = cpool.tile([P, KB, K_ATOMS], f16)
            mir = mi.rearrange("(kb p) m -> p kb m", p=P)
            nc.sync.dma_start(mi_sb[:, 0:2, :], mir[:, 0:2, :])
            nc.sync.dma_start(mi_sb[:, 2:4, :], mir[:, 2:4, :])
            rs_sb = cpool.tile([P, KB], f32)
            nc.sync.dma_start(rs_sb[:], rs[:])
            id_sb = cpool.tile([P, P], f16)
            nc.sync.dma_start(id_sb[:], ident[:])
            gb_sb = cpool.tile([P, KB, BS], f32)

            outr = out.rearrange("(mb p) n -> p mb n", p=P)

            w_cur = [None] * KB
            s01 = [None, None]  # classic chunks' S tiles (f32)
            s23 = None  # ident chunks' S_comb pair tile (f16, [P, 2, BS])

            def mm_slice(pair, m):
                return pair[:, (m % 2) * BS : (m % 2 + 1) * BS]

            # ---- iteration 1: Gb = Ht_aug^T @ Yt_aug (m-outer blocks) ----
            pgs = [
                ppool.tile([P, 2 * BS], f32, tag="pp", name=f"pg{h}") for h in range(2)
            ]
            for m in range(KB):
                for kb in range(3):
                    nc.tensor.matmul(
                        mm_slice(pgs[m // 2], m),
                        lhsT=ht_sb[:, kb, m * P : (m + 1) * P],
                        rhs=yt_sb[:, kb, :],
                        start=(kb == 0),
                        stop=(kb == 2),
                    )
                pslice = mm_slice(pgs[m // 2], m)
                if niter == 1:
                    xm = opool.tile([P, BS], f32, tag="x", name=f"x1{m}")
                    nc.vector.tensor_scalar_add(xm[:], pslice, rs_sb[:, m : m + 1])
                    nc.sync.dma_start(outr[:, m, :], xm[:])
                else:
                    # v_1 = G = Gb + t*rowsum(Minv)
                    vm = vpool.tile([P, BS], f32, tag="v", name=f"v1{m}")
                    nc.vector.tensor_scalar_add(vm[:], pslice, rs_sb[:, m : m + 1])
                    wm = wpool.tile([P, BS], f16, tag="w", name=f"w1{m}")
                    nc.scalar.activation(wm[:], vm[:], Act.Abs, bias=nb[:, 0:1])
                    # Gb to SBUF (needed every iteration)
                    nc.scalar.activation(gb_sb[:, m, :], pslice, Act.Copy)
                    w_cur[m] = wm
                    if m < 2:
                        sm = spool.tile([P, BS], f32, tag=f"s{m}", name=f"s1{m}")
                        nc.vector.scalar_tensor_tensor(
                            sm[:], vm[:], THR, gb_sb[:, m, :], Alu.min, Alu.add
                        )
                        s01[m] = sm
                    else:
                        if s23 is None:
                            s23 = spool.tile([P, 2, BS], f16, tag="s23", name="s23_1")
                        nc.vector.scalar_tensor_tensor(
                            s23[:, m - 2, :], vm[:], THR, gb_sb[:, m, :],
                            Alu.min, Alu.add,
                        )

            # ---- iterations 2..niter ----
            for it in range(2, niter + 1):
                last = it == niter
                pps = [
                    ppool.tile([P, 2 * BS], f32, tag="pp", name=f"pp{it}_{h}")
                    for h in range(2)
                ]
                vs = [None, None]
                neww = [None] * KB
                news01 = [None, None]
                for m in range(KB):
                    use_ident = (m >= 2) and not last
                    pslice = mm_slice(pps[m // 2], m)
                    if use_ident:
                        # v' accumulates directly in PSUM: I @ S_comb + Minv @ w
                        nc.tensor.matmul(
                            pslice,
                            lhsT=id_sb[:],
                            rhs=s23[:, m - 2, :],
                            start=True,
                            stop=False,
                        )
                    for kb in range(KB):
                        nc.tensor.matmul(
                            pslice,
                            lhsT=mi_sb[:, kb, m * P : (m + 1) * P],
                            rhs=w_cur[kb][:],
                            start=(kb == 0) and not use_ident,
                            stop=(kb == KB - 1),
                        )
                    if last:
                        if m % 2 == 1:
                            xp = opool.tile([P, 2, BS], f32, tag="x", name=f"x{m}")
                            nc.vector.scalar_tensor_tensor(
                                xp[:],
                                pps[m // 2][:],
                                0.0,
                                gb_sb[:, m - 1 : m + 1, :],
                                Alu.bypass,
                                Alu.add,
                            )
                            nc.sync.dma_start(outr[:, m - 1 : m + 1, :], xp[:])
                    elif m < 2:
                        # V-op: v = psum + S_prev (critical chain)
                        vm = vpool.tile([P, BS], f32, tag="v", name=f"v{it}_{m}")
                        nc.vector.scalar_tensor_tensor(
                            vm[:], pslice, 0.0, s01[m][:], Alu.bypass, Alu.add
                        )
                        vs[m] = vm
                        wm = wpool.tile([P, BS], f16, tag="w", name=f"w{it}_{m}")
                        nc.scalar.activation(wm[:], vm[:], Act.Abs, bias=nb[:, 0:1])
                        neww[m] = wm
                    else:
                        # v lives in PSUM; ACT reads it directly
                        wm = wpool.tile([P, BS], f16, tag="w", name=f"w{it}_{m}")
                        nc.scalar.activation(wm[:], pslice, Act.Abs, bias=nb[:, 0:1])
                        neww[m] = wm
                if not last:
                    # S_comb for chunks 2,3 as one wide op over the PSUM pair
                    ns23 = spool.tile([P, 2, BS], f16, tag="s23", name=f"s23_{it}")
                    nc.vector.scalar_tensor_tensor(
                        ns23[:], pps[1][:], THR, gb_sb[:, 2:4, :], Alu.min, Alu.add
                    )
                    for m in range(2):
                        sm = spool.tile([P, BS], f32, tag=f"s{m}", name=f"s{it}_{m}")
                        nc.vector.scalar_tensor_tensor(
                            sm[:], vs[m][:], THR, gb_sb[:, m, :], Alu.min, Alu.add
                        )
                        news01[m] = sm
                    w_cur, s01, s23 = neww, news01, ns23

    nc.finalize()
    return nc


def _get_nc(niter):
    if niter not in _NC_CACHE:
        _NC_CACHE[niter] = _build(niter)
    return _NC_CACHE[niter]


def _prep_in_maps(Y, A):
    """Host precompute of the A-derived (voxel-independent) factor matrices,
    in float64: the inverse replaces the reference's Cholesky solve. Shards Y
    over voxels (transposed, with the augmented ones-row appended)."""
    A64 = A.astype(np.float64)
    LHS = A64.T @ A64 + RHO * np.eye(K_ATOMS)
    Minv = np.linalg.inv(LHS)
    Minv = (Minv + Minv.T) / 2
    Hm = A64 @ Minv  # [M, K]
    rsum = Minv.sum(axis=1)

    Ht = np.zeros((3 * P, K_ATOMS), np.float16)
    Ht[:M_MEAS] = Hm.astype(np.float16)
    Ht[M_MEAS] = (-THR * rsum).astype(np.float16)
    Mi = Minv.astype(np.float16)
    rs = np.ascontiguousarray((THR * rsum).astype(np.float32).reshape(KB, P).T)
    Id = np.eye(P, dtype=np.float16)

    in_maps = []
    for c in range(N_CORES):
        Yt = np.zeros((3 * P, BS), np.float16)
        Yt[:M_MEAS] = Y[c * BS : (c + 1) * BS, :].T.astype(np.float16)
        Yt[M_MEAS] = 1.0
        in_maps.append(
            {"Yt": np.ascontiguousarray(Yt), "Ht": Ht, "Mi": Mi, "rs": rs, "Id": Id}
        )
    return in_maps


def kernel(Y, A, max_iter):
    from concourse.bass_utils import run_bass_kernel_spmd

    Y = np.ascontiguousarray(np.asarray(Y, dtype=np.float32))
    A = np.ascontiguousarray(np.asarray(A, dtype=np.float32))
    niter = int(max_iter)
    assert Y.shape == (B_VOX, M_MEAS) and A.shape == (M_MEAS, K_ATOMS)
    assert niter >= 1

    in_maps = _prep_in_maps(Y, A)
    nc = _get_nc(niter)
    res = run_bass_kernel_spmd(nc, in_maps, core_ids=list(range(N_CORES)))

    outp = np.empty((B_VOX, K_ATOMS), np.float32)
    for c in range(N_CORES):
        outp[c * BS : (c + 1) * BS] = res.results[c]["out"].T
    return outp


# revision 8
# speedup vs baseline: 1.4439x; 1.4439x over previous
"""AMICO ADMM solver on 8 Trainium2 NeuronCores.

Problem: X = argmin ||Y^T - A x||^2 + lam*||x||_1 s.t. x >= 0, solved with
max_iter ADMM steps (rho=1, lam=0.1) exactly as in the reference scan.

Algebraic reduction (tracking only v = x + u):
    v_1 = G                      with G  = Minv @ A^T @ Y^T
    for i = 2..N:
        w   = |v - t|            (t = lam/rho)
        S   = min(v, t) + Gb     (Gb = G - t * Minv @ 1)
        v'  = Minv @ w + S
    output x_N = Minv @ w_{N-1} + Gb

since z = relu(v - t), u' = v - z = min(v, t), and z - u' = |v - t| - t.
The constant -t*Minv@1 and the A^T Y^T term are folded into a single
"augmented" matmul: Gb = Ht_aug^T @ Yt_aug where Ht_aug carries A@Minv plus a
bias row (-t * rowsum(Minv)) and Yt_aug carries Y^T plus a row of ones.

Sharding: data-parallel over voxels (B=4096 -> 512 per core); A-derived
matrices (Minv, Ht_aug) replicated; no cross-core communication.

Implementation notes (measured on silicon):
 - All matmul operands are fp16 (11-bit mantissa; fp32/fp16 mixing is
   rejected by the compiler, bf16 weights lose too much accuracy).
   End-to-end error vs the float32 cho_solve reference: ~6e-3.
 - Output chunks 0,1 use a DVE V-op (v = psum + S) with fp16 v/S state so
   the S op hits the DVE 16-bit 2x mode; chunks 2,3 instead accumulate
   I @ S_comb into the PSUM group via an identity matmul, so v materializes
   directly in PSUM and the Abs activation reads it from there, shortening
   the cross-iteration chain and balancing PE vs DVE work.
"""

import numpy as np

B_VOX = 4096
M_MEAS = 256
K_ATOMS = 512
P = 128
N_CORES = 8
BS = B_VOX // N_CORES  # 512 voxels per core
KB = K_ATOMS // P  # 4 chunks of the contraction/output dim
LAM = 0.1
RHO = 1.0
THR = LAM / RHO

_NC_CACHE = {}


def _build(niter):
    import concourse.mybir as mybir
    import concourse.tile as tile
    from concourse import bacc

    f32 = mybir.dt.float32
    f16 = mybir.dt.float16
    Alu = mybir.AluOpType
    Act = mybir.ActivationFunctionType

    nc = bacc.Bacc(None, target_bir_lowering=False)
    ht = nc.declare_dram_parameter("Ht", [3 * P, K_ATOMS], f16, isOutput=False)
    yt = nc.declare_dram_parameter("Yt", [3 * P, BS], f16, isOutput=False)
    mi = nc.declare_dram_parameter("Mi", [K_ATOMS, K_ATOMS], f16, isOutput=False)
    rs = nc.declare_dram_parameter("rs", [P, KB], f32, isOutput=False)
    ident = nc.declare_dram_parameter("Id", [P, P], f16, isOutput=False)
    out = nc.declare_dram_parameter("out", [K_ATOMS, BS], f32, isOutput=True)

    with tile.TileContext(nc) as tc:
        with (
            tc.tile_pool(name="const", bufs=1) as cpool,
            tc.tile_pool(name="v", bufs=8) as vpool,
            tc.tile_pool(name="w", bufs=12) as wpool,
            tc.tile_pool(name="s", bufs=8) as spool,
            tc.tile_pool(name="o", bufs=4) as opool,
            tc.tile_pool(name="psum", bufs=8, space="PSUM") as ppool,
        ):
            nb = cpool.tile([P, 1], f32)
            nc.vector.memset(nb[:], -THR)
            # split the big loads -> parallel DMA queues
            ht_sb = cpool.tile([P, 3, K_ATOMS], f16)
            htr = ht.rearrange("(kb p) a -> p kb a", p=P)
            nc.sync.dma_start(ht_sb[:, 0:2, :], htr[:, 0:2, :])
            nc.sync.dma_start(ht_sb[:, 2:3, :], htr[:, 2:3, :])
            yt_sb = cpool.tile([P, 3, BS], f16)
            ytr = yt.rearrange("(kb p) b -> p kb b", p=P)
            nc.sync.dma_start(yt_sb[:, 0:2, :], ytr[:, 0:2, :])
            nc.sync.dma_start(yt_sb[:, 2:3, :], ytr[:, 2:3, :])
            mi_sb = cpool.tile([P, KB, K_ATOMS], f16)
            mir = mi.rearrange("(kb p) m -> p kb m", p=P)
            nc.sync.dma_start(mi_sb[:, 0:2, :], mir[:, 0:2, :])
            nc.sync.dma_start(mi_sb[:, 2:4, :], mir[:, 2:4, :])
            rs_sb = cpool.tile([P, KB], f32)
            nc.sync.dma_start(rs_sb[:], rs[:])
            id_sb = cpool.tile([P, P], f16)
            nc.sync.dma_start(id_sb[:], ident[:])
            gb_sb = cpool.tile([P, KB, BS], f32)
            gb16_sb = cpool.tile([P, 2, BS], f16)  # fp16 copy for chunks 0,1 S-op

            outr = out.rearrange("(mb p) n -> p mb n", p=P)

            w_cur = [None] * KB
            s_cur = [None] * KB  # chunks 0,1: S (f16); chunks 2,3: S_comb (f16)

            # ---- iteration 1: Gb = Ht_aug^T @ Yt_aug (m-outer blocks) ----
            pgs = [
                ppool.tile([P, BS], f32, tag="pp", name=f"pg{m}") for m in range(KB)
            ]
            for m in range(KB):
                for kb in range(3):
                    nc.tensor.matmul(
                        pgs[m][:],
                        lhsT=ht_sb[:, kb, m * P : (m + 1) * P],
                        rhs=yt_sb[:, kb, :],
                        start=(kb == 0),
                        stop=(kb == 2),
                    )
                if niter == 1:
                    xm = opool.tile([P, BS], f32, tag="x", name=f"x1{m}")
                    nc.vector.tensor_scalar_add(xm[:], pgs[m][:], rs_sb[:, m : m + 1])
                    nc.sync.dma_start(outr[:, m, :], xm[:])
                else:
                    # v_1 = G = Gb + t*rowsum(Minv)
                    vm = vpool.tile([P, BS], f16, tag="v", name=f"v1{m}")
                    nc.vector.tensor_scalar_add(vm[:], pgs[m][:], rs_sb[:, m : m + 1])
                    wm = wpool.tile([P, BS], f16, tag="w", name=f"w1{m}")
                    nc.scalar.activation(wm[:], vm[:], Act.Abs, bias=nb[:, 0:1])
                    # Gb to SBUF (needed every iteration)
                    nc.scalar.activation(gb_sb[:, m, :], pgs[m][:], Act.Copy)
                    if m < 2:
                        nc.scalar.activation(gb16_sb[:, m, :], pgs[m][:], Act.Copy)
                    sm = spool.tile([P, BS], f16, tag=f"s{m}", name=f"s1{m}")
                    gbin = gb16_sb[:, m, :] if m < 2 else gb_sb[:, m, :]
                    nc.vector.scalar_tensor_tensor(
                        sm[:], vm[:], THR, gbin, Alu.min, Alu.add
                    )
                    w_cur[m], s_cur[m] = wm, sm

            # ---- iterations 2..niter ----
            for it in range(2, niter + 1):
                last = it == niter
                pps = [
                    ppool.tile([P, BS], f32, tag="pp", name=f"pp{it}_{m}")
                    for m in range(KB)
                ]
                vs = [None, None]
                neww = [None] * KB
                news = [None] * KB
                for m in range(KB):
                    use_ident = (m >= 2) and not last
                    if use_ident:
                        # v' accumulates directly in PSUM: I @ S_comb + Minv @ w
                        nc.tensor.matmul(
                            pps[m][:],
                            lhsT=id_sb[:],
                            rhs=s_cur[m][:],
                            start=True,
                            stop=False,
                        )
                    for kb in range(KB):
                        nc.tensor.matmul(
                            pps[m][:],
                            lhsT=mi_sb[:, kb, m * P : (m + 1) * P],
                            rhs=w_cur[kb][:],
                            start=(kb == 0) and not use_ident,
                            stop=(kb == KB - 1),
                        )
                    if last:
                        xm = opool.tile([P, BS], f32, tag="x", name=f"x{m}")
                        nc.vector.scalar_tensor_tensor(
                            xm[:], pps[m][:], 0.0, gb_sb[:, m, :], Alu.bypass, Alu.add
                        )
                        nc.sync.dma_start(outr[:, m, :], xm[:])
                    elif m < 2:
                        # V-op: v = psum + S_prev (critical chain)
                        vm = vpool.tile([P, BS], f16, tag="v", name=f"v{it}_{m}")
                        nc.vector.scalar_tensor_tensor(
                            vm[:], pps[m][:], 0.0, s_cur[m][:], Alu.bypass, Alu.add
                        )
                        vs[m] = vm
                        wm = wpool.tile([P, BS], f16, tag="w", name=f"w{it}_{m}")
                        nc.scalar.activation(wm[:], vm[:], Act.Abs, bias=nb[:, 0:1])
                        neww[m] = wm
                    else:
                        # v lives in PSUM; ACT reads it directly
                        wm = wpool.tile([P, BS], f16, tag="w", name=f"w{it}_{m}")
                        nc.scalar.activation(wm[:], pps[m][:], Act.Abs, bias=nb[:, 0:1])
                        neww[m] = wm
                        sm = spool.tile([P, BS], f16, tag=f"s{m}", name=f"s{it}_{m}")
                        nc.vector.scalar_tensor_tensor(
                            sm[:], pps[m][:], THR, gb_sb[:, m, :], Alu.min, Alu.add
                        )
                        news[m] = sm
                if not last:
                    # S ops for chunks 0,1: fp16 x fp16 -> DVE 2x mode
                    for m in range(2):
                        sm = spool.tile([P, BS], f16, tag=f"s{m}", name=f"s{it}_{m}")
                        nc.vector.scalar_tensor_tensor(
                            sm[:], vs[m][:], THR, gb16_sb[:, m, :], Alu.min, Alu.add
                        )
                        news[m] = sm
                    w_cur, s_cur = neww, news

    nc.finalize()
    return nc


def _get_nc(niter):
    if niter not in _NC_CACHE:
        _NC_CACHE[niter] = _build(niter)
    return _NC_CACHE[niter]


def _prep_in_maps(Y, A):
    """Host precompute of the A-derived (voxel-independent) factor matrices,
    in float64: the inverse replaces the reference's Cholesky solve. Shards Y
    over voxels (transposed, with the augmented ones-row appended)."""
    A64 = A.astype(np.float64)
    LHS = A64.T @ A64 + RHO * np.eye(K_ATOMS)
    Minv = np.linalg.inv(LHS)
    Minv = (Minv + Minv.T) / 2
    Hm = A64 @ Minv  # [M, K]
    rsum = Minv.sum(axis=1)

    Ht = np.zeros((3 * P, K_ATOMS), np.float16)
    Ht[:M_MEAS] = Hm.astype(np.float16)
    Ht[M_MEAS] = (-THR * rsum).astype(np.float16)
    Mi = Minv.astype(np.float16)
    rs = np.ascontiguousarray((THR * rsum).astype(np.float32).reshape(KB, P).T)
    Id = np.eye(P, dtype=np.float16)

    in_maps = []
    for c in range(N_CORES):
        Yt = np.zeros((3 * P, BS), np.float16)
        Yt[:M_MEAS] = Y[c * BS : (c + 1) * BS, :].T.astype(np.float16)
        Yt[M_MEAS] = 1.0
        in_maps.append(
            {"Yt": np.ascontiguousarray(Yt), "Ht": Ht, "Mi": Mi, "rs": rs, "Id": Id}
        )
    return in_maps


def kernel(Y, A, max_iter):
    from concourse.bass_utils import run_bass_kernel_spmd

    Y = np.ascontiguousarray(np.asarray(Y, dtype=np.float32))
    A = np.ascontiguousarray(np.asarray(A, dtype=np.float32))
    niter = int(max_iter)
    assert Y.shape == (B_VOX, M_MEAS) and A.shape == (M_MEAS, K_ATOMS)
    assert niter >= 1

    in_maps = _prep_in_maps(Y, A)
    nc = _get_nc(niter)
    res = run_bass_kernel_spmd(nc, in_maps, core_ids=list(range(N_CORES)))

    outp = np.empty((B_VOX, K_ATOMS), np.float32)
    for c in range(N_CORES):
        outp[c * BS : (c + 1) * BS] = res.results[c]["out"].T
    return outp


# revision 10
# speedup vs baseline: 1.6432x; 1.1381x over previous
"""AMICO ADMM solver on 8 Trainium2 NeuronCores.

Problem: X = argmin ||Y^T - A x||^2 + lam*||x||_1 s.t. x >= 0, solved with
max_iter ADMM steps (rho=1, lam=0.1) exactly as in the reference scan.

Algebraic reduction (tracking only v = x + u):
    v_1 = G                      with G  = Minv @ A^T @ Y^T
    for i = 2..N:
        w   = |v - t|            (t = lam/rho)
        S   = min(v, t) + Gb     (Gb = G - t * Minv @ 1)
        v'  = Minv @ w + S
    output x_N = Minv @ w_{N-1} + Gb

since z = relu(v - t), u' = v - z = min(v, t), and z - u' = |v - t| - t.
The constant -t*Minv@1 and the A^T Y^T term are folded into a single
"augmented" matmul: Gb = Ht_aug^T @ Yt_aug where Ht_aug carries A@Minv plus a
bias row (-t * rowsum(Minv)) and Yt_aug carries Y^T plus a row of ones.

Sharding: data-parallel over voxels (B=4096 -> 512 per core); A-derived
matrices (Minv, Ht_aug) replicated; no cross-core communication.

Implementation notes (measured on silicon):
 - All matmul operands are fp16 (11-bit mantissa; fp32/fp16 mixing is
   rejected by the compiler, bf16 weights lose too much accuracy).
   End-to-end error vs the float32 cho_solve reference: ~6e-3.
 - Output chunks 0,1 use a DVE V-op (v = psum + S) with fp16 v/S state so
   the S op hits the DVE 16-bit 2x mode; chunks 2,3 instead accumulate
   I @ S_comb into the PSUM group via an identity matmul, so v materializes
   directly in PSUM and the Abs activation reads it from there, shortening
   the cross-iteration chain and balancing PE vs DVE work.
"""

import numpy as np

B_VOX = 4096
M_MEAS = 256
K_ATOMS = 512
P = 128
N_CORES = 8
BS = B_VOX // N_CORES  # 512 voxels per core
KB = K_ATOMS // P  # 4 chunks of the contraction/output dim
LAM = 0.1
RHO = 1.0
THR = LAM / RHO

_NC_CACHE = {}


def _build(niter):
    import concourse.mybir as mybir
    import concourse.tile as tile
    from concourse import bacc

    f32 = mybir.dt.float32
    f16 = mybir.dt.float16
    Alu = mybir.AluOpType
    Act = mybir.ActivationFunctionType

    nc = bacc.Bacc(None, target_bir_lowering=False)
    ht = nc.declare_dram_parameter("Ht", [3 * P, K_ATOMS], f16, isOutput=False)
    yt = nc.declare_dram_parameter("Yt", [3 * P, BS], f16, isOutput=False)
    mi = nc.declare_dram_parameter("Mi", [K_ATOMS, K_ATOMS], f16, isOutput=False)
    rs = nc.declare_dram_parameter("rs", [P, KB], f32, isOutput=False)
    ident = nc.declare_dram_parameter("Id", [P, P], f16, isOutput=False)
    out = nc.declare_dram_parameter("out", [K_ATOMS, BS], f32, isOutput=True)

    with tile.TileContext(nc) as tc:
        with (
            tc.tile_pool(name="const", bufs=1) as cpool,
            tc.tile_pool(name="v", bufs=8) as vpool,
            tc.tile_pool(name="w", bufs=12) as wpool,
            tc.tile_pool(name="s", bufs=8) as spool,
            tc.tile_pool(name="o", bufs=4) as opool,
            tc.tile_pool(name="psum", bufs=8, space="PSUM") as ppool,
        ):
            nb = cpool.tile([P, 1], f32)
            nc.vector.memset(nb[:], -THR)
            # split the big loads -> parallel DMA queues
            ht_sb = cpool.tile([P, 3, K_ATOMS], f16)
            htr = ht.rearrange("(kb p) a -> p kb a", p=P)
            for _c in range(3):
                nc.sync.dma_start(ht_sb[:, _c : _c + 1, :], htr[:, _c : _c + 1, :])
            yt_sb = cpool.tile([P, 3, BS], f16)
            ytr = yt.rearrange("(kb p) b -> p kb b", p=P)
            for _c in range(3):
                nc.sync.dma_start(yt_sb[:, _c : _c + 1, :], ytr[:, _c : _c + 1, :])
            mi_sb = cpool.tile([P, KB, K_ATOMS], f16)
            mir = mi.rearrange("(kb p) m -> p kb m", p=P)
            for _c in range(4):
                nc.sync.dma_start(mi_sb[:, _c : _c + 1, :], mir[:, _c : _c + 1, :])
            rs_sb = cpool.tile([P, KB], f32)
            nc.sync.dma_start(rs_sb[:], rs[:])
            id_sb = cpool.tile([P, P], f16)
            nc.sync.dma_start(id_sb[:], ident[:])
            gb_sb = cpool.tile([P, KB, BS], f32)
            gb16_sb = cpool.tile([P, 2, BS], f16)  # fp16 copy for chunks 0,1 S-op

            outr = out.rearrange("(mb p) n -> p mb n", p=P)

            w_cur = [None] * KB
            s_cur = [None] * KB  # chunks 0,1: S (f16); chunks 2,3: S_comb (f16)

            # ---- iteration 1: Gb = Ht_aug^T @ Yt_aug (m-outer blocks) ----
            pgs = [
                ppool.tile([P, BS], f32, tag="pp", name=f"pg{m}") for m in range(KB)
            ]
            for m in range(KB):
                for kb in range(3):
                    nc.tensor.matmul(
                        pgs[m][:],
                        lhsT=ht_sb[:, kb, m * P : (m + 1) * P],
                        rhs=yt_sb[:, kb, :],
                        start=(kb == 0),
                        stop=(kb == 2),
                    )
                if niter == 1:
                    xm = opool.tile([P, BS], f32, tag="x", name=f"x1{m}")
                    nc.vector.tensor_scalar_add(xm[:], pgs[m][:], rs_sb[:, m : m + 1])
                    nc.sync.dma_start(outr[:, m, :], xm[:])
                else:
                    # v_1 = G = Gb + t*rowsum(Minv)
                    vm = vpool.tile([P, BS], f16, tag="v", name=f"v1{m}")
                    nc.vector.tensor_scalar_add(vm[:], pgs[m][:], rs_sb[:, m : m + 1])
                    wm = wpool.tile([P, BS], f16, tag="w", name=f"w1{m}")
                    nc.scalar.activation(wm[:], vm[:], Act.Abs, bias=nb[:, 0:1])
                    # Gb to SBUF (needed every iteration)
                    nc.scalar.activation(gb_sb[:, m, :], pgs[m][:], Act.Copy)
                    if m < 2:
                        nc.scalar.activation(gb16_sb[:, m, :], pgs[m][:], Act.Copy)
                    sm = spool.tile([P, BS], f16, tag=f"s{m}", name=f"s1{m}")
                    gbin = gb16_sb[:, m, :] if m < 2 else gb_sb[:, m, :]
                    nc.vector.scalar_tensor_tensor(
                        sm[:], vm[:], THR, gbin, Alu.min, Alu.add
                    )
                    w_cur[m], s_cur[m] = wm, sm

            # ---- iterations 2..niter ----
            for it in range(2, niter + 1):
                last = it == niter
                pps = [
                    ppool.tile([P, BS], f32, tag="pp", name=f"pp{it}_{m}")
                    for m in range(KB)
                ]
                vs = [None, None]
                neww = [None] * KB
                news = [None] * KB
                for m in range(KB):
                    use_ident = (m >= 2) and not last
                    if use_ident:
                        # v' accumulates directly in PSUM: I @ S_comb + Minv @ w
                        nc.tensor.matmul(
                            pps[m][:],
                            lhsT=id_sb[:],
                            rhs=s_cur[m][:],
                            start=True,
                            stop=False,
                        )
                    for kb in range(KB):
                        nc.tensor.matmul(
                            pps[m][:],
                            lhsT=mi_sb[:, kb, m * P : (m + 1) * P],
                            rhs=w_cur[kb][:],
                            start=(kb == 0) and not use_ident,
                            stop=(kb == KB - 1),
                        )
                    if last:
                        xm = opool.tile([P, BS], f32, tag="x", name=f"x{m}")
                        nc.vector.scalar_tensor_tensor(
                            xm[:], pps[m][:], 0.0, gb_sb[:, m, :], Alu.bypass, Alu.add
                        )
                        nc.sync.dma_start(outr[:, m, 0 : BS // 2], xm[:, 0 : BS // 2])
                        nc.sync.dma_start(outr[:, m, BS // 2 :], xm[:, BS // 2 :])
                    elif m < 2:
                        # V-op: v = psum + S_prev (critical chain)
                        vm = vpool.tile([P, BS], f16, tag="v", name=f"v{it}_{m}")
                        nc.vector.scalar_tensor_tensor(
                            vm[:], pps[m][:], 0.0, s_cur[m][:], Alu.bypass, Alu.add
                        )
                        vs[m] = vm
                        wm = wpool.tile([P, BS], f16, tag="w", name=f"w{it}_{m}")
                        nc.scalar.activation(wm[:], vm[:], Act.Abs, bias=nb[:, 0:1])
                        neww[m] = wm
                    else:
                        # v lives in PSUM; ACT reads it directly
                        wm = wpool.tile([P, BS], f16, tag="w", name=f"w{it}_{m}")
                        nc.scalar.activation(wm[:], pps[m][:], Act.Abs, bias=nb[:, 0:1])
                        neww[m] = wm
                        sm = spool.tile([P, BS], f16, tag=f"s{m}", name=f"s{it}_{m}")
                        nc.vector.scalar_tensor_tensor(
                            sm[:], pps[m][:], THR, gb_sb[:, m, :], Alu.min, Alu.add
                        )
                        news[m] = sm
                if not last:
                    # S ops for chunks 0,1 (off the critical chain)
                    for m in range(2):
                        sm = spool.tile([P, BS], f16, tag=f"s{m}", name=f"s{it}_{m}")
                        nc.vector.scalar_tensor_tensor(
                            sm[:], vs[m][:], THR, gb16_sb[:, m, :], Alu.min, Alu.add
                        )
                        news[m] = sm
                    w_cur, s_cur = neww, news

    nc.finalize()
    return nc


def _get_nc(niter):
    if niter not in _NC_CACHE:
        _NC_CACHE[niter] = _build(niter)
    return _NC_CACHE[niter]


def _prep_in_maps(Y, A):
    """Host precompute of the A-derived (voxel-independent) factor matrices,
    in float64: the inverse replaces the reference's Cholesky solve. Shards Y
    over voxels (transposed, with the augmented ones-row appended)."""
    A64 = A.astype(np.float64)
    LHS = A64.T @ A64 + RHO * np.eye(K_ATOMS)
    Minv = np.linalg.inv(LHS)
    Minv = (Minv + Minv.T) / 2
    Hm = A64 @ Minv  # [M, K]
    rsum = Minv.sum(axis=1)

    Ht = np.zeros((3 * P, K_ATOMS), np.float16)
    Ht[:M_MEAS] = Hm.astype(np.float16)
    Ht[M_MEAS] = (-THR * rsum).astype(np.float16)
    Mi = Minv.astype(np.float16)
    rs = np.ascontiguousarray((THR * rsum).astype(np.float32).reshape(KB, P).T)
    Id = np.eye(P, dtype=np.float16)

    in_maps = []
    for c in range(N_CORES):
        Yt = np.zeros((3 * P, BS), np.float16)
        Yt[:M_MEAS] = Y[c * BS : (c + 1) * BS, :].T.astype(np.float16)
        Yt[M_MEAS] = 1.0
        in_maps.append(
            {"Yt": np.ascontiguousarray(Yt), "Ht": Ht, "Mi": Mi, "rs": rs, "Id": Id}
        )
    return in_maps


def kernel(Y, A, max_iter):
    from concourse.bass_utils import run_bass_kernel_spmd

    Y = np.ascontiguousarray(np.asarray(Y, dtype=np.float32))
    A = np.ascontiguousarray(np.asarray(A, dtype=np.float32))
    niter = int(max_iter)
    assert Y.shape == (B_VOX, M_MEAS) and A.shape == (M_MEAS, K_ATOMS)
    assert niter >= 1

    in_maps = _prep_in_maps(Y, A)
    nc = _get_nc(niter)
    res = run_bass_kernel_spmd(nc, in_maps, core_ids=list(range(N_CORES)))

    outp = np.empty((B_VOX, K_ATOMS), np.float32)
    for c in range(N_CORES):
        outp[c * BS : (c + 1) * BS] = res.results[c]["out"].T
    return outp


# revision 12
# speedup vs baseline: 1.6680x; 1.0150x over previous
"""AMICO ADMM solver on 8 Trainium2 NeuronCores.

Problem: X = argmin ||Y^T - A x||^2 + lam*||x||_1 s.t. x >= 0, solved with
max_iter ADMM steps (rho=1, lam=0.1) exactly as in the reference scan.

Algebraic reduction (tracking only v = x + u):
    v_1 = G                      with G  = Minv @ A^T @ Y^T
    for i = 2..N:
        w   = |v - t|            (t = lam/rho)
        S   = min(v, t) + Gb     (Gb = G - t * Minv @ 1)
        v'  = Minv @ w + S
    output x_N = Minv @ w_{N-1} + Gb

since z = relu(v - t), u' = v - z = min(v, t), and z - u' = |v - t| - t.
The constant -t*Minv@1 and the A^T Y^T term are folded into a single
"augmented" matmul: Gb = Ht_aug^T @ Yt_aug where Ht_aug carries A@Minv plus a
bias row (-t * rowsum(Minv)) and Yt_aug carries Y^T plus a row of ones.

Sharding: data-parallel over voxels (B=4096 -> 512 per core); A-derived
matrices (Minv, Ht_aug) replicated; no cross-core communication.

Implementation notes (measured on silicon):
 - All matmul operands are fp16 (11-bit mantissa; fp32/fp16 mixing is
   rejected by the compiler, bf16 weights lose too much accuracy).
   End-to-end error vs the float32 cho_solve reference: ~6e-3.
 - Output chunks 0,1 use a DVE V-op (v = psum + S) with fp16 v/S state so
   the S op hits the DVE 16-bit 2x mode; chunks 2,3 instead accumulate
   I @ S_comb into the PSUM group via an identity matmul, so v materializes
   directly in PSUM and the Abs activation reads it from there, shortening
   the cross-iteration chain and balancing PE vs DVE work.
"""

import numpy as np

B_VOX = 4096
M_MEAS = 256
K_ATOMS = 512
P = 128
N_CORES = 8
BS = B_VOX // N_CORES  # 512 voxels per core
KB = K_ATOMS // P  # 4 chunks of the contraction/output dim
LAM = 0.1
RHO = 1.0
THR = LAM / RHO

_NC_CACHE = {}


def _build(niter):
    import concourse.mybir as mybir
    import concourse.tile as tile
    from concourse import bacc

    f32 = mybir.dt.float32
    f16 = mybir.dt.float16
    Alu = mybir.AluOpType
    Act = mybir.ActivationFunctionType

    nc = bacc.Bacc(None, target_bir_lowering=False)
    # one host-pre-transposed packed param: per partition p the row holds
    # [Ht (3*512) | Yt (3*512) | Mi (4*512) | Id (128) | rs (4)] in fp16,
    # so every DMA descriptor is a multi-KB contiguous run.
    NPACK = 3 * K_ATOMS + 3 * BS + KB * K_ATOMS + P + KB
    packed = nc.declare_dram_parameter("packed", [P, NPACK], f16, isOutput=False)
    out = nc.declare_dram_parameter("out", [K_ATOMS, BS], f32, isOutput=True)
    HT0 = 0
    YT0 = 3 * K_ATOMS
    MI0 = YT0 + 3 * BS
    NMI = KB * K_ATOMS + P + KB  # mi + id + rs tail in one tile

    with tile.TileContext(nc) as tc:
        with (
            tc.tile_pool(name="const", bufs=1) as cpool,
            tc.tile_pool(name="v", bufs=8) as vpool,
            tc.tile_pool(name="w", bufs=12) as wpool,
            tc.tile_pool(name="s", bufs=8) as spool,
            tc.tile_pool(name="o", bufs=4) as opool,
            tc.tile_pool(name="psum", bufs=8, space="PSUM") as ppool,
        ):
            nb = cpool.tile([P, 1], f32)
            nc.vector.memset(nb[:], -THR)
            # parallel large-descriptor loads from the packed param
            ht_sb = cpool.tile([P, 3 * K_ATOMS], f16)
            yt_sb = cpool.tile([P, 3 * BS], f16)
            for _c in range(3):
                _a, _b = _c * K_ATOMS, (_c + 1) * K_ATOMS
                nc.sync.dma_start(ht_sb[:, _a:_b], packed[:, HT0 + _a : HT0 + _b])
                nc.sync.dma_start(yt_sb[:, _a:_b], packed[:, YT0 + _a : YT0 + _b])
            mi_sb = cpool.tile([P, NMI], f16)
            _h = NMI // 2
            nc.sync.dma_start(mi_sb[:, 0:_h], packed[:, MI0 : MI0 + _h])
            nc.sync.dma_start(mi_sb[:, _h:], packed[:, MI0 + _h :])
            id_sb = mi_sb[:, KB * K_ATOMS : KB * K_ATOMS + P]
            rs_sb = cpool.tile([P, KB], f32)
            nc.vector.tensor_copy(rs_sb[:], mi_sb[:, KB * K_ATOMS + P :])
            gb_sb = cpool.tile([P, KB, BS], f32)
            gb16_sb = cpool.tile([P, 2, BS], f16)  # fp16 copy for chunks 0,1 S-op

            outr = out.rearrange("(mb p) n -> p mb n", p=P)

            w_cur = [None] * KB
            s_cur = [None] * KB  # chunks 0,1: S (f16); chunks 2,3: S_comb (f16)

            # ---- iteration 1: Gb = Ht_aug^T @ Yt_aug (m-outer blocks) ----
            pgs = [
                ppool.tile([P, BS], f32, tag="pp", name=f"pg{m}") for m in range(KB)
            ]
            for kb in range(3):
                for m in range(KB):
                    nc.tensor.matmul(
                        pgs[m][:],
                        lhsT=ht_sb[:, kb * K_ATOMS + m * P : kb * K_ATOMS + (m + 1) * P],
                        rhs=yt_sb[:, kb * BS : (kb + 1) * BS],
                        start=(kb == 0),
                        stop=(kb == 2),
                    )
            for m in range(KB):
                if niter == 1:
                    xm = opool.tile([P, BS], f32, tag="x", name=f"x1{m}")
                    nc.vector.tensor_scalar_add(xm[:], pgs[m][:], rs_sb[:, m : m + 1])
                    nc.sync.dma_start(outr[:, m, :], xm[:])
                else:
                    # v_1 = G = Gb + t*rowsum(Minv)
                    vm = vpool.tile([P, BS], f16, tag="v", name=f"v1{m}")
                    nc.vector.tensor_scalar_add(vm[:], pgs[m][:], rs_sb[:, m : m + 1])
                    wm = wpool.tile([P, BS], f16, tag="w", name=f"w1{m}")
                    nc.scalar.activation(wm[:], vm[:], Act.Abs, bias=nb[:, 0:1])
                    # Gb to SBUF (needed every iteration)
                    nc.scalar.activation(gb_sb[:, m, :], pgs[m][:], Act.Copy)
                    if m < 2:
                        nc.scalar.activation(gb16_sb[:, m, :], pgs[m][:], Act.Copy)
                    sm = spool.tile([P, BS], f16, tag=f"s{m}", name=f"s1{m}")
                    gbin = gb16_sb[:, m, :] if m < 2 else gb_sb[:, m, :]
                    nc.vector.scalar_tensor_tensor(
                        sm[:], vm[:], THR, gbin, Alu.min, Alu.add
                    )
                    w_cur[m], s_cur[m] = wm, sm

            # ---- iterations 2..niter ----
            for it in range(2, niter + 1):
                last = it == niter
                pps = [
                    ppool.tile([P, BS], f32, tag="pp", name=f"pp{it}_{m}")
                    for m in range(KB)
                ]
                vs = [None, None]
                neww = [None] * KB
                news = [None] * KB
                for m in range(KB):
                    use_ident = (m >= 2) and not last
                    if use_ident:
                        # v' accumulates directly in PSUM: I @ S_comb + Minv @ w
                        nc.tensor.matmul(
                            pps[m][:],
                            lhsT=id_sb[:],
                            rhs=s_cur[m][:],
                            start=True,
                            stop=False,
                        )
                    for kb in range(KB):
                        nc.tensor.matmul(
                            pps[m][:],
                            lhsT=mi_sb[:, kb * K_ATOMS + m * P : kb * K_ATOMS + (m + 1) * P],
                            rhs=w_cur[kb][:],
                            start=(kb == 0) and not use_ident,
                            stop=(kb == KB - 1),
                        )
                    if last:
                        xm = opool.tile([P, BS], f32, tag="x", name=f"x{m}")
                        nc.vector.scalar_tensor_tensor(
                            xm[:], pps[m][:], 0.0, gb_sb[:, m, :], Alu.bypass, Alu.add
                        )
                        nc.sync.dma_start(outr[:, m, 0 : BS // 2], xm[:, 0 : BS // 2])
                        nc.sync.dma_start(outr[:, m, BS // 2 :], xm[:, BS // 2 :])
                    elif m < 2:
                        # V-op: v = psum + S_prev (critical chain)
                        vm = vpool.tile([P, BS], f16, tag="v", name=f"v{it}_{m}")
                        nc.vector.scalar_tensor_tensor(
                            vm[:], pps[m][:], 0.0, s_cur[m][:], Alu.bypass, Alu.add
                        )
                        vs[m] = vm
                        wm = wpool.tile([P, BS], f16, tag="w", name=f"w{it}_{m}")
                        nc.scalar.activation(wm[:], vm[:], Act.Abs, bias=nb[:, 0:1])
                        neww[m] = wm
                    else:
                        # v lives in PSUM; ACT reads it directly
                        wm = wpool.tile([P, BS], f16, tag="w", name=f"w{it}_{m}")
                        nc.scalar.activation(wm[:], pps[m][:], Act.Abs, bias=nb[:, 0:1])
                        neww[m] = wm
                        sm = spool.tile([P, BS], f16, tag=f"s{m}", name=f"s{it}_{m}")
                        nc.vector.scalar_tensor_tensor(
                            sm[:], pps[m][:], THR, gb_sb[:, m, :], Alu.min, Alu.add
                        )
                        news[m] = sm
                if not last:
                    # S ops for chunks 0,1 (off the critical chain)
                    for m in range(2):
                        sm = spool.tile([P, BS], f16, tag=f"s{m}", name=f"s{it}_{m}")
                        nc.vector.scalar_tensor_tensor(
                            sm[:], vs[m][:], THR, gb16_sb[:, m, :], Alu.min, Alu.add
                        )
                        news[m] = sm
                    w_cur, s_cur = neww, news

    nc.finalize()
    return nc


def _get_nc(niter):
    if niter not in _NC_CACHE:
        _NC_CACHE[niter] = _build(niter)
    return _NC_CACHE[niter]


def _prep_in_maps(Y, A):
    """Host precompute of the A-derived (voxel-independent) factor matrices,
    in float64: the inverse replaces the reference's Cholesky solve. Shards Y
    over voxels (transposed + augmented ones-row) and packs all device inputs
    into one pre-transposed [128, NPACK] fp16 array so every DMA descriptor
    is a multi-KB contiguous run."""
    A64 = A.astype(np.float64)
    LHS = A64.T @ A64 + RHO * np.eye(K_ATOMS)
    Minv = np.linalg.inv(LHS)
    Minv = (Minv + Minv.T) / 2
    Hm = A64 @ Minv  # [M, K]
    rsum = Minv.sum(axis=1)

    Ht = np.zeros((3 * P, K_ATOMS), np.float16)
    Ht[:M_MEAS] = Hm.astype(np.float16)
    Ht[M_MEAS] = (-THR * rsum).astype(np.float16)
    # -> [P, 3*K]: htp[p, kb*K + a] = Ht[kb*P + p, a]
    htp = Ht.reshape(3, P, K_ATOMS).transpose(1, 0, 2).reshape(P, 3 * K_ATOMS)
    Mi = Minv.astype(np.float16)
    mip = Mi.reshape(KB, P, K_ATOMS).transpose(1, 0, 2).reshape(P, KB * K_ATOMS)
    rs = (THR * rsum).astype(np.float16).reshape(KB, P).T  # [P, KB]
    Id = np.eye(P, dtype=np.float16)
    fixed = np.concatenate([mip, Id, rs], axis=1)  # [P, KB*K + P + KB]

    in_maps = []
    for c in range(N_CORES):
        Yt = np.zeros((3 * P, BS), np.float16)
        Yt[:M_MEAS] = Y[c * BS : (c + 1) * BS, :].T.astype(np.float16)
        Yt[M_MEAS] = 1.0
        ytp = Yt.reshape(3, P, BS).transpose(1, 0, 2).reshape(P, 3 * BS)
        packed = np.ascontiguousarray(
            np.concatenate([htp, ytp, fixed], axis=1)
        )
        in_maps.append({"packed": packed})
    return in_maps


def kernel(Y, A, max_iter):
    from concourse.bass_utils import run_bass_kernel_spmd

    Y = np.ascontiguousarray(np.asarray(Y, dtype=np.float32))
    A = np.ascontiguousarray(np.asarray(A, dtype=np.float32))
    niter = int(max_iter)
    assert Y.shape == (B_VOX, M_MEAS) and A.shape == (M_MEAS, K_ATOMS)
    assert niter >= 1

    in_maps = _prep_in_maps(Y, A)
    nc = _get_nc(niter)
    res = run_bass_kernel_spmd(nc, in_maps, core_ids=list(range(N_CORES)))

    outp = np.empty((B_VOX, K_ATOMS), np.float32)
    for c in range(N_CORES):
        outp[c * BS : (c + 1) * BS] = res.results[c]["out"].T
    return outp


# revision 13
# speedup vs baseline: 1.6686x; 1.0004x over previous
"""AMICO ADMM solver on 8 Trainium2 NeuronCores.

Problem: X = argmin ||Y^T - A x||^2 + lam*||x||_1 s.t. x >= 0, solved with
max_iter ADMM steps (rho=1, lam=0.1) exactly as in the reference scan.

Algebraic reduction (tracking only v = x + u):
    v_1 = G                      with G  = Minv @ A^T @ Y^T
    for i = 2..N:
        w   = |v - t|            (t = lam/rho)
        S   = min(v, t) + Gb     (Gb = G - t * Minv @ 1)
        v'  = Minv @ w + S
    output x_N = Minv @ w_{N-1} + Gb

since z = relu(v - t), u' = v - z = min(v, t), and z - u' = |v - t| - t.
The constant -t*Minv@1 and the A^T Y^T term are folded into a single
"augmented" matmul: Gb = Ht_aug^T @ Yt_aug where Ht_aug carries A@Minv plus a
bias row (-t * rowsum(Minv)) and Yt_aug carries Y^T plus a row of ones.

Sharding: data-parallel over voxels (B=4096 -> 512 per core); A-derived
matrices (Minv, Ht_aug) replicated; no cross-core communication.

Implementation notes (measured on silicon):
 - All matmul operands are fp16 (11-bit mantissa; fp32/fp16 mixing is
   rejected by the compiler, bf16 weights lose too much accuracy).
   End-to-end error vs the float32 cho_solve reference: ~6e-3.
 - Output chunks 0,1 use a DVE V-op (v = psum + S) with fp16 v/S state so
   the S op hits the DVE 16-bit 2x mode; chunks 2,3 instead accumulate
   I @ S_comb into the PSUM group via an identity matmul, so v materializes
   directly in PSUM and the Abs activation reads it from there, shortening
   the cross-iteration chain and balancing PE vs DVE work.
"""

import numpy as np

B_VOX = 4096
M_MEAS = 256
K_ATOMS = 512
P = 128
N_CORES = 8
BS = B_VOX // N_CORES  # 512 voxels per core
KB = K_ATOMS // P  # 4 chunks of the contraction/output dim
LAM = 0.1
RHO = 1.0
THR = LAM / RHO

_NC_CACHE = {}


def _build(niter):
    import concourse.mybir as mybir
    import concourse.tile as tile
    from concourse import bacc

    f32 = mybir.dt.float32
    f16 = mybir.dt.float16
    Alu = mybir.AluOpType
    Act = mybir.ActivationFunctionType

    nc = bacc.Bacc(None, target_bir_lowering=False)
    # one host-pre-transposed packed param: per partition p the row holds
    # [Ht_kb0|Yt_kb0|Ht_kb1|Yt_kb1|Ht_kb2|Yt_kb2 | Id | rs | Mi] in fp16,
    # so every DMA descriptor is a multi-KB contiguous run and each G-round's
    # operands arrive in a single transfer.
    NHY = 3 * (K_ATOMS + BS)
    NMI = P + KB + KB * K_ATOMS  # id + rs + mi in one tile
    NPACK = NHY + NMI
    packed = nc.declare_dram_parameter("packed", [P, NPACK], f16, isOutput=False)
    out = nc.declare_dram_parameter("out", [K_ATOMS, BS], f32, isOutput=True)
    MI0 = NHY
    MIW = P + KB  # weight columns start here inside mi_sb

    with tile.TileContext(nc) as tc:
        with (
            tc.tile_pool(name="const", bufs=1) as cpool,
            tc.tile_pool(name="v", bufs=8) as vpool,
            tc.tile_pool(name="w", bufs=12) as wpool,
            tc.tile_pool(name="s", bufs=8) as spool,
            tc.tile_pool(name="o", bufs=4) as opool,
            tc.tile_pool(name="psum", bufs=8, space="PSUM") as ppool,
        ):
            nb = cpool.tile([P, 1], f32)
            nc.vector.memset(nb[:], -THR)
            # parallel large-descriptor loads from the packed param
            hy_sb = cpool.tile([P, NHY], f16)
            _kbw = K_ATOMS + BS
            for _c in range(3):
                nc.sync.dma_start(
                    hy_sb[:, _c * _kbw : (_c + 1) * _kbw],
                    packed[:, _c * _kbw : (_c + 1) * _kbw],
                )
            mi_sb = cpool.tile([P, NMI], f16)
            _h = NMI // 2
            nc.sync.dma_start(mi_sb[:, 0:_h], packed[:, MI0 : MI0 + _h])
            nc.sync.dma_start(mi_sb[:, _h:], packed[:, MI0 + _h :])
            id_sb = mi_sb[:, 0:P]
            rs_sb = cpool.tile([P, KB], f32)
            nc.vector.tensor_copy(rs_sb[:], mi_sb[:, P : P + KB])
            gb_sb = cpool.tile([P, KB, BS], f32)
            gb16_sb = cpool.tile([P, 2, BS], f16)  # fp16 copy for chunks 0,1 S-op

            outr = out.rearrange("(mb p) n -> p mb n", p=P)

            w_cur = [None] * KB
            s_cur = [None] * KB  # chunks 0,1: S (f16); chunks 2,3: S_comb (f16)

            # ---- iteration 1: Gb = Ht_aug^T @ Yt_aug (m-outer blocks) ----
            pgs = [
                ppool.tile([P, BS], f32, tag="pp", name=f"pg{m}") for m in range(KB)
            ]
            for kb in range(3):
                for m in range(KB):
                    nc.tensor.matmul(
                        pgs[m][:],
                        lhsT=hy_sb[:, kb * _kbw + m * P : kb * _kbw + (m + 1) * P],
                        rhs=hy_sb[:, kb * _kbw + K_ATOMS : (kb + 1) * _kbw],
                        start=(kb == 0),
                        stop=(kb == 2),
                    )
            for m in range(KB):
                if niter == 1:
                    xm = opool.tile([P, BS], f32, tag="x", name=f"x1{m}")
                    nc.vector.tensor_scalar_add(xm[:], pgs[m][:], rs_sb[:, m : m + 1])
                    nc.sync.dma_start(outr[:, m, :], xm[:])
                else:
                    # v_1 = G = Gb + t*rowsum(Minv)
                    vm = vpool.tile([P, BS], f16, tag="v", name=f"v1{m}")
                    nc.vector.tensor_scalar_add(vm[:], pgs[m][:], rs_sb[:, m : m + 1])
                    wm = wpool.tile([P, BS], f16, tag="w", name=f"w1{m}")
                    nc.scalar.activation(wm[:], vm[:], Act.Abs, bias=nb[:, 0:1])
                    # Gb to SBUF (needed every iteration)
                    nc.scalar.activation(gb_sb[:, m, :], pgs[m][:], Act.Copy)
                    if m < 2:
                        nc.scalar.activation(gb16_sb[:, m, :], pgs[m][:], Act.Copy)
                    sm = spool.tile([P, BS], f16, tag=f"s{m}", name=f"s1{m}")
                    gbin = gb16_sb[:, m, :] if m < 2 else gb_sb[:, m, :]
                    nc.vector.scalar_tensor_tensor(
                        sm[:], vm[:], THR, gbin, Alu.min, Alu.add
                    )
                    w_cur[m], s_cur[m] = wm, sm

            # ---- iterations 2..niter ----
            for it in range(2, niter + 1):
                last = it == niter
                pps = [
                    ppool.tile([P, BS], f32, tag="pp", name=f"pp{it}_{m}")
                    for m in range(KB)
                ]
                vs = [None, None]
                neww = [None] * KB
                news = [None] * KB
                for m in range(KB):
                    use_ident = (m >= 2) and not last
                    if use_ident:
                        # v' accumulates directly in PSUM: I @ S_comb + Minv @ w
                        nc.tensor.matmul(
                            pps[m][:],
                            lhsT=id_sb[:],
                            rhs=s_cur[m][:],
                            start=True,
                            stop=False,
                        )
                    for kb in range(KB):
                        nc.tensor.matmul(
                            pps[m][:],
                            lhsT=mi_sb[:, MIW + kb * K_ATOMS + m * P : MIW + kb * K_ATOMS + (m + 1) * P],
                            rhs=w_cur[kb][:],
                            start=(kb == 0) and not use_ident,
                            stop=(kb == KB - 1),
                        )
                    if last:
                        xm = opool.tile([P, BS], f32, tag="x", name=f"x{m}")
                        nc.vector.scalar_tensor_tensor(
                            xm[:], pps[m][:], 0.0, gb_sb[:, m, :], Alu.bypass, Alu.add
                        )
                        nc.sync.dma_start(outr[:, m, 0 : BS // 2], xm[:, 0 : BS // 2])
                        nc.sync.dma_start(outr[:, m, BS // 2 :], xm[:, BS // 2 :])
                    elif m < 2:
                        # V-op: v = psum + S_prev (critical chain)
                        vm = vpool.tile([P, BS], f16, tag="v", name=f"v{it}_{m}")
                        nc.vector.scalar_tensor_tensor(
                            vm[:], pps[m][:], 0.0, s_cur[m][:], Alu.bypass, Alu.add
                        )
                        vs[m] = vm
                        wm = wpool.tile([P, BS], f16, tag="w", name=f"w{it}_{m}")
                        nc.scalar.activation(wm[:], vm[:], Act.Abs, bias=nb[:, 0:1])
                        neww[m] = wm
                    else:
                        # v lives in PSUM; ACT reads it directly
                        wm = wpool.tile([P, BS], f16, tag="w", name=f"w{it}_{m}")
                        nc.scalar.activation(wm[:], pps[m][:], Act.Abs, bias=nb[:, 0:1])
                        neww[m] = wm
                        sm = spool.tile([P, BS], f16, tag=f"s{m}", name=f"s{it}_{m}")
                        nc.vector.scalar_tensor_tensor(
                            sm[:], pps[m][:], THR, gb_sb[:, m, :], Alu.min, Alu.add
                        )
                        news[m] = sm
                if not last:
                    # S ops for chunks 0,1 (off the critical chain)
                    for m in range(2):
                        sm = spool.tile([P, BS], f16, tag=f"s{m}", name=f"s{it}_{m}")
                        nc.vector.scalar_tensor_tensor(
                            sm[:], vs[m][:], THR, gb16_sb[:, m, :], Alu.min, Alu.add
                        )
                        news[m] = sm
                    w_cur, s_cur = neww, news

    nc.finalize()
    return nc


def _get_nc(niter):
    if niter not in _NC_CACHE:
        _NC_CACHE[niter] = _build(niter)
    return _NC_CACHE[niter]


def _prep_in_maps(Y, A):
    """Host precompute of the A-derived (voxel-independent) factor matrices,
    in float64: the inverse replaces the reference's Cholesky solve. Shards Y
    over voxels (transposed + augmented ones-row) and packs all device inputs
    into one pre-transposed [128, NPACK] fp16 array so every DMA descriptor
    is a multi-KB contiguous run."""
    A64 = A.astype(np.float64)
    LHS = A64.T @ A64 + RHO * np.eye(K_ATOMS)
    Minv = np.linalg.inv(LHS)
    Minv = (Minv + Minv.T) / 2
    Hm = A64 @ Minv  # [M, K]
    rsum = Minv.sum(axis=1)

    Ht = np.zeros((3 * P, K_ATOMS), np.float16)
    Ht[:M_MEAS] = Hm.astype(np.float16)
    Ht[M_MEAS] = (-THR * rsum).astype(np.float16)
    # -> [P, 3*K]: htp[p, kb*K + a] = Ht[kb*P + p, a]
    htp = Ht.reshape(3, P, K_ATOMS).transpose(1, 0, 2)  # [P, 3, K]
    Mi = Minv.astype(np.float16)
    mip = Mi.reshape(KB, P, K_ATOMS).transpose(1, 0, 2).reshape(P, KB * K_ATOMS)
    rs = (THR * rsum).astype(np.float16).reshape(KB, P).T  # [P, KB]
    Id = np.eye(P, dtype=np.float16)
    fixed = np.concatenate([Id, rs, mip], axis=1)  # [P, P + KB + KB*K]

    in_maps = []
    for c in range(N_CORES):
        Yt = np.zeros((3 * P, BS), np.float16)
        Yt[:M_MEAS] = Y[c * BS : (c + 1) * BS, :].T.astype(np.float16)
        Yt[M_MEAS] = 1.0
        ytp = Yt.reshape(3, P, BS).transpose(1, 0, 2)  # [P, 3, BS]
        hy = np.concatenate([htp, ytp], axis=2).reshape(P, 3 * (K_ATOMS + BS))
        packed = np.ascontiguousarray(np.concatenate([hy, fixed], axis=1))
        in_maps.append({"packed": packed})
    return in_maps


def kernel(Y, A, max_iter):
    from concourse.bass_utils import run_bass_kernel_spmd

    Y = np.ascontiguousarray(np.asarray(Y, dtype=np.float32))
    A = np.ascontiguousarray(np.asarray(A, dtype=np.float32))
    niter = int(max_iter)
    assert Y.shape == (B_VOX, M_MEAS) and A.shape == (M_MEAS, K_ATOMS)
    assert niter >= 1

    in_maps = _prep_in_maps(Y, A)
    nc = _get_nc(niter)
    res = run_bass_kernel_spmd(nc, in_maps, core_ids=list(range(N_CORES)))

    outp = np.empty((B_VOX, K_ATOMS), np.float32)
    for c in range(N_CORES):
        outp[c * BS : (c + 1) * BS] = res.results[c]["out"].T
    return outp


# revision 16
# speedup vs baseline: 1.6764x; 1.0047x over previous
"""AMICO ADMM solver on 8 Trainium2 NeuronCores.

Problem: X = argmin ||Y^T - A x||^2 + lam*||x||_1 s.t. x >= 0, solved with
max_iter ADMM steps (rho=1, lam=0.1) exactly as in the reference scan.

Algebraic reduction (tracking only v = x + u):
    v_1 = G                      with G  = Minv @ A^T @ Y^T
    for i = 2..N:
        w   = |v - t|            (t = lam/rho)
        S   = min(v, t) + Gb     (Gb = G - t * Minv @ 1)
        v'  = Minv @ w + S
    output x_N = Minv @ w_{N-1} + Gb

since z = relu(v - t), u' = v - z = min(v, t), and z - u' = |v - t| - t.
The constant -t*Minv@1 and the A^T Y^T term are folded into a single
"augmented" matmul: Gb = Ht_aug^T @ Yt_aug where Ht_aug carries A@Minv plus a
bias row (-t * rowsum(Minv)) and Yt_aug carries Y^T plus a row of ones.

Sharding: data-parallel over voxels (B=4096 -> 512 per core); A-derived
matrices (Minv, Ht_aug) replicated; no cross-core communication.

Implementation notes (measured on silicon):
 - All matmul operands are fp16 (11-bit mantissa; fp32/fp16 mixing is
   rejected by the compiler, bf16 weights lose too much accuracy).
   End-to-end error vs the float32 cho_solve reference: ~6e-3.
 - Output chunks 0,1 use a DVE V-op (v = psum + S) with fp16 v/S state so
   the S op hits the DVE 16-bit 2x mode; chunks 2,3 instead accumulate
   I @ S_comb into the PSUM group via an identity matmul, so v materializes
   directly in PSUM and the Abs activation reads it from there, shortening
   the cross-iteration chain and balancing PE vs DVE work.
"""

import numpy as np

B_VOX = 4096
M_MEAS = 256
K_ATOMS = 512
P = 128
N_CORES = 8
BS = B_VOX // N_CORES  # 512 voxels per core
KB = K_ATOMS // P  # 4 chunks of the contraction/output dim
LAM = 0.1
RHO = 1.0
THR = LAM / RHO

_NC_CACHE = {}


def _build(niter):
    import concourse.mybir as mybir
    import concourse.tile as tile
    from concourse import bacc

    f32 = mybir.dt.float32
    f16 = mybir.dt.float16
    Alu = mybir.AluOpType
    Act = mybir.ActivationFunctionType

    nc = bacc.Bacc(None, target_bir_lowering=False)
    # one host-pre-transposed packed param: per partition p the row holds
    # [Ht_kb0|Yt_kb0|Ht_kb1|Yt_kb1|Ht_kb2|Yt_kb2 | Id | rs | Mi] in fp16,
    # so every DMA descriptor is a multi-KB contiguous run and each G-round's
    # operands arrive in a single transfer.
    NHY = 3 * (K_ATOMS + BS)
    NMI = P + KB + KB * K_ATOMS  # id + rs + mi in one tile
    NPACK = NHY + NMI
    packed = nc.declare_dram_parameter("packed", [P, NPACK], f16, isOutput=False)
    out = nc.declare_dram_parameter("out", [K_ATOMS, BS], f32, isOutput=True)
    MI0 = NHY
    MIW = P + KB  # weight columns start here inside mi_sb

    with tile.TileContext(nc) as tc:
        with (
            tc.tile_pool(name="const", bufs=1) as cpool,
            tc.tile_pool(name="v", bufs=8) as vpool,
            tc.tile_pool(name="w", bufs=12) as wpool,
            tc.tile_pool(name="s", bufs=8) as spool,
            tc.tile_pool(name="o", bufs=4) as opool,
            tc.tile_pool(name="psum", bufs=8, space="PSUM") as ppool,
        ):
            nb = cpool.tile([P, 1], f32)
            nc.vector.memset(nb[:], -THR)
            # parallel large-descriptor loads from the packed param
            hy_sb = cpool.tile([P, NHY], f16)
            _kbw = K_ATOMS + BS
            for _c in range(3):
                nc.sync.dma_start(
                    hy_sb[:, _c * _kbw : (_c + 1) * _kbw],
                    packed[:, _c * _kbw : (_c + 1) * _kbw],
                )
            mi_sb = cpool.tile([P, NMI], f16)
            _h = NMI // 2
            nc.sync.dma_start(mi_sb[:, 0:_h], packed[:, MI0 : MI0 + _h])
            nc.sync.dma_start(mi_sb[:, _h:], packed[:, MI0 + _h :])
            id_sb = mi_sb[:, 0:P]
            rs_sb = cpool.tile([P, KB], f32)
            nc.vector.tensor_copy(rs_sb[:], mi_sb[:, P : P + KB])
            gb_sb = cpool.tile([P, KB, BS], f32)
            gb16_sb = cpool.tile([P, 2, BS], f16)  # fp16 copy for chunks 0,1 S-op

            outr = out.rearrange("(mb p) n -> p mb n", p=P)

            w_cur = [None] * KB
            s_cur = [None] * KB  # chunks 0,1: S (f16); chunks 2,3: S_comb (f16)

            # ---- iteration 1: Gb = Ht_aug^T @ Yt_aug (m-outer blocks) ----
            pgs = [
                ppool.tile([P, BS], f32, tag="pp", name=f"pg{m}") for m in range(KB)
            ]
            for kb in range(3):
                for m in range(KB):
                    nc.tensor.matmul(
                        pgs[m][:],
                        lhsT=hy_sb[:, kb * _kbw + m * P : kb * _kbw + (m + 1) * P],
                        rhs=hy_sb[:, kb * _kbw + K_ATOMS : (kb + 1) * _kbw],
                        start=(kb == 0),
                        stop=(kb == 2),
                    )
            for m in range(KB):
                if niter == 1:
                    xm = opool.tile([P, BS], f32, tag="x", name=f"x1{m}")
                    nc.vector.tensor_scalar_add(xm[:], pgs[m][:], rs_sb[:, m : m + 1])
                    nc.sync.dma_start(outr[:, m, :], xm[:])
                else:
                    # v_1 = G = Gb + t*rowsum(Minv)
                    vm = vpool.tile([P, BS], f16, tag="v", name=f"v1{m}")
                    nc.vector.tensor_scalar_add(vm[:], pgs[m][:], rs_sb[:, m : m + 1])
                    wm = wpool.tile([P, BS], f16, tag="w", name=f"w1{m}")
                    nc.scalar.activation(wm[:], vm[:], Act.Abs, bias=nb[:, 0:1])
                    # Gb to SBUF (needed every iteration)
                    nc.scalar.activation(gb_sb[:, m, :], pgs[m][:], Act.Copy)
                    if m < 2:
                        nc.scalar.activation(gb16_sb[:, m, :], pgs[m][:], Act.Copy)
                    sm = spool.tile([P, BS], f16, tag=f"s{m}", name=f"s1{m}")
                    gbin = gb16_sb[:, m, :] if m < 2 else gb_sb[:, m, :]
                    nc.vector.scalar_tensor_tensor(
                        sm[:], vm[:], THR, gbin, Alu.min, Alu.add
                    )
                    w_cur[m], s_cur[m] = wm, sm

            # ---- iterations 2..niter ----
            for it in range(2, niter + 1):
                last = it == niter
                pps = [
                    ppool.tile([P, BS], f32, tag="pp", name=f"pp{it}_{m}")
                    for m in range(KB)
                ]
                vs = [None, None]
                neww = [None] * KB
                news = [None] * KB
                for m in range(KB):
                    use_ident = (m >= 2) and not last
                    if use_ident:
                        # v' accumulates directly in PSUM: I @ S_comb + Minv @ w
                        nc.tensor.matmul(
                            pps[m][:],
                            lhsT=id_sb[:],
                            rhs=s_cur[m][:],
                            start=True,
                            stop=False,
                        )
                    for kb in range(KB):
                        nc.tensor.matmul(
                            pps[m][:],
                            lhsT=mi_sb[:, MIW + kb * K_ATOMS + m * P : MIW + kb * K_ATOMS + (m + 1) * P],
                            rhs=w_cur[kb][:],
                            start=(kb == 0) and not use_ident,
                            stop=(kb == KB - 1),
                        )
                    if last:
                        xm = opool.tile([P, BS], f32, tag="x", name=f"x{m}")
                        nc.vector.scalar_tensor_tensor(
                            xm[:], pps[m][:], 0.0, gb_sb[:, m, :], Alu.bypass, Alu.add
                        )
                        nc.sync.dma_start(outr[:, m, 0 : BS // 2], xm[:, 0 : BS // 2])
                        nc.sync.dma_start(outr[:, m, BS // 2 :], xm[:, BS // 2 :])
                    elif m < 2:
                        # V-op: v = psum + S_prev (critical chain)
                        vm = vpool.tile([P, BS], f16, tag="v", name=f"v{it}_{m}")
                        nc.vector.scalar_tensor_tensor(
                            vm[:], pps[m][:], 0.0, s_cur[m][:], Alu.bypass, Alu.add
                        )
                        vs[m] = vm
                        wm = wpool.tile([P, BS], f16, tag="w", name=f"w{it}_{m}")
                        nc.scalar.activation(wm[:], vm[:], Act.Abs, bias=nb[:, 0:1])
                        neww[m] = wm
                    else:
                        # v lives in PSUM; ACT reads it directly
                        wm = wpool.tile([P, BS], f16, tag="w", name=f"w{it}_{m}")
                        nc.scalar.activation(wm[:], pps[m][:], Act.Abs, bias=nb[:, 0:1])
                        neww[m] = wm
                        sm = spool.tile([P, BS], f16, tag=f"s{m}", name=f"s{it}_{m}")
                        nc.vector.scalar_tensor_tensor(
                            sm[:], pps[m][:], THR, gb_sb[:, m, :], Alu.min, Alu.add
                        )
                        news[m] = sm
                if not last:
                    # S ops for chunks 0,1 (off the critical chain)
                    for m in range(2):
                        sm = spool.tile([P, BS], f16, tag=f"s{m}", name=f"s{it}_{m}")
                        nc.vector.scalar_tensor_tensor(
                            sm[:], vs[m][:], THR, gb16_sb[:, m, :], Alu.min, Alu.add
                        )
                        news[m] = sm
                    w_cur, s_cur = neww, news

    nc.finalize()
    return nc


def _get_nc(niter):
    if niter not in _NC_CACHE:
        _NC_CACHE[niter] = _build(niter)
    return _NC_CACHE[niter]


def _prep_in_maps(Y, A):
    """Host precompute of the A-derived (voxel-independent) factor matrices,
    in float64: the inverse replaces the reference's Cholesky solve. Shards Y
    over voxels (transposed + augmented ones-row) and packs all device inputs
    into one pre-transposed [128, NPACK] fp16 array so every DMA descriptor
    is a multi-KB contiguous run."""
    A64 = A.astype(np.float64)
    LHS = A64.T @ A64 + RHO * np.eye(K_ATOMS)
    Minv = np.linalg.inv(LHS)
    Minv = (Minv + Minv.T) / 2
    Hm = A64 @ Minv  # [M, K]
    rsum = Minv.sum(axis=1)

    Ht = np.zeros((3 * P, K_ATOMS), np.float16)
    Ht[:M_MEAS] = Hm.astype(np.float16)
    Ht[M_MEAS] = (-THR * rsum).astype(np.float16)
    # -> [P, 3*K]: htp[p, kb*K + a] = Ht[kb*P + p, a]
    htp = Ht.reshape(3, P, K_ATOMS).transpose(1, 0, 2)  # [P, 3, K]
    Mi = Minv.astype(np.float16)
    mip = Mi.reshape(KB, P, K_ATOMS).transpose(1, 0, 2).reshape(P, KB * K_ATOMS)
    rs = (THR * rsum).astype(np.float16).reshape(KB, P).T  # [P, KB]
    Id = np.eye(P, dtype=np.float16)
    fixed = np.concatenate([Id, rs, mip], axis=1)  # [P, P + KB + KB*K]

    in_maps = []
    for c in range(N_CORES):
        Yt = np.zeros((3 * P, BS), np.float16)
        Yt[:M_MEAS] = Y[c * BS : (c + 1) * BS, :].T.astype(np.float16)
        Yt[M_MEAS] = 1.0
        ytp = Yt.reshape(3, P, BS).transpose(1, 0, 2)  # [P, 3, BS]
        hy = np.concatenate([htp, ytp], axis=2).reshape(P, 3 * (K_ATOMS + BS))
        packed = np.ascontiguousarray(np.concatenate([hy, fixed], axis=1))
        in_maps.append({"packed": packed})
    return in_maps


def kernel(Y, A, max_iter):
    from concourse.bass_utils import run_bass_kernel_spmd

    Y = np.ascontiguousarray(np.asarray(Y, dtype=np.float32))
    A = np.ascontiguousarray(np.asarray(A, dtype=np.float32))
    niter = int(max_iter)
    assert Y.shape == (B_VOX, M_MEAS) and A.shape == (M_MEAS, K_ATOMS)
    assert niter >= 1

    in_maps = _prep_in_maps(Y, A)
    nc = _get_nc(niter)
    res = run_bass_kernel_spmd(nc, in_maps, core_ids=list(range(N_CORES)))

    outp = np.empty((B_VOX, K_ATOMS), np.float32)
    for c in range(N_CORES):
        outp[c * BS : (c + 1) * BS] = res.results[c]["out"].T
    return outp


# revision 17
# speedup vs baseline: 1.6947x; 1.0109x over previous
"""AMICO ADMM solver on 8 Trainium2 NeuronCores.

Problem: X = argmin ||Y^T - A x||^2 + lam*||x||_1 s.t. x >= 0, solved with
max_iter ADMM steps (rho=1, lam=0.1) exactly as in the reference scan.

Algebraic reduction (tracking only v = x + u):
    v_1 = G                      with G  = Minv @ A^T @ Y^T
    for i = 2..N:
        w   = |v - t|            (t = lam/rho)
        S   = min(v, t) + Gb     (Gb = G - t * Minv @ 1)
        v'  = Minv @ w + S
    output x_N = Minv @ w_{N-1} + Gb

since z = relu(v - t), u' = v - z = min(v, t), and z - u' = |v - t| - t.
The constant -t*Minv@1 and the A^T Y^T term are folded into a single
"augmented" matmul: Gb = Ht_aug^T @ Yt_aug where Ht_aug carries A@Minv plus a
bias row (-t * rowsum(Minv)) and Yt_aug carries Y^T plus a row of ones.

Sharding: data-parallel over voxels (B=4096 -> 512 per core); A-derived
matrices (Minv, Ht_aug) replicated; no cross-core communication.

Implementation notes (measured on silicon):
 - All matmul operands are fp16 (11-bit mantissa; fp32/fp16 mixing is
   rejected by the compiler, bf16 weights lose too much accuracy).
   End-to-end error vs the float32 cho_solve reference: ~6e-3.
 - Output chunks 0,1 use a DVE V-op (v = psum + S) with fp16 v/S state so
   the S op hits the DVE 16-bit 2x mode; chunks 2,3 instead accumulate
   I @ S_comb into the PSUM group via an identity matmul, so v materializes
   directly in PSUM and the Abs activation reads it from there, shortening
   the cross-iteration chain and balancing PE vs DVE work.
"""

import numpy as np

B_VOX = 4096
M_MEAS = 256
K_ATOMS = 512
P = 128
N_CORES = 8
BS = B_VOX // N_CORES  # 512 voxels per core
KB = K_ATOMS // P  # 4 chunks of the contraction/output dim
LAM = 0.1
RHO = 1.0
THR = LAM / RHO

_NC_CACHE = {}


def _build(niter):
    import concourse.mybir as mybir
    import concourse.tile as tile
    from concourse import bacc

    f32 = mybir.dt.float32
    f16 = mybir.dt.float16
    Alu = mybir.AluOpType
    Act = mybir.ActivationFunctionType

    nc = bacc.Bacc(None, target_bir_lowering=False)
    # one host-pre-transposed packed param: per partition p the row holds
    # [Ht_kb0|Yt_kb0|Ht_kb1|Yt_kb1|Ht_kb2|Yt_kb2 | Id | rs | Mi] in fp16,
    # so every DMA descriptor is a multi-KB contiguous run and each G-round's
    # operands arrive in a single transfer.
    NHY = 2 * (K_ATOMS + BS)
    NMI = P + KB + KB * K_ATOMS  # id + cneg + mi in one tile
    NPACK = NHY + NMI
    packed = nc.declare_dram_parameter("packed", [P, NPACK], f16, isOutput=False)
    out = nc.declare_dram_parameter("out", [K_ATOMS, BS], f32, isOutput=True)
    MI0 = NHY
    MIW = P + KB  # weight columns start here inside mi_sb

    with tile.TileContext(nc) as tc:
        with (
            tc.tile_pool(name="const", bufs=1) as cpool,
            tc.tile_pool(name="v", bufs=8) as vpool,
            tc.tile_pool(name="w", bufs=12) as wpool,
            tc.tile_pool(name="s", bufs=8) as spool,
            tc.tile_pool(name="o", bufs=4) as opool,
            tc.tile_pool(name="psum", bufs=8, space="PSUM") as ppool,
        ):
            nb = cpool.tile([P, 1], f32)
            nc.vector.memset(nb[:], -THR)
            # parallel large-descriptor loads from the packed param
            hy_sb = cpool.tile([P, NHY], f16)
            _kbw = K_ATOMS + BS
            for _c in range(2):
                nc.sync.dma_start(
                    hy_sb[:, _c * _kbw : (_c + 1) * _kbw],
                    packed[:, _c * _kbw : (_c + 1) * _kbw],
                )
            mi_sb = cpool.tile([P, NMI], f16)
            _h = NMI // 2
            nc.sync.dma_start(mi_sb[:, 0:_h], packed[:, MI0 : MI0 + _h])
            nc.sync.dma_start(mi_sb[:, _h:], packed[:, MI0 + _h :])
            id_sb = mi_sb[:, 0:P]
            cn_sb = cpool.tile([P, KB], f32)
            nc.vector.tensor_copy(cn_sb[:], mi_sb[:, P : P + KB])
            gb_sb = cpool.tile([P, KB, BS], f32)
            gb16_sb = cpool.tile([P, 2, BS], f16)  # fp16 copy for chunks 0,1 S-op

            outr = out.rearrange("(mb p) n -> p mb n", p=P)

            w_cur = [None] * KB
            s_cur = [None] * KB  # chunks 0,1: S (f16); chunks 2,3: S_comb (f16)

            # ---- iteration 1: G = H^T @ Yt (m-outer blocks); v_1 = G stays
            # in PSUM and Gb = G - t*rowsum(Minv) is applied as a per-
            # partition ACT bias during the copy to SBUF. ----
            pgs = [
                ppool.tile([P, BS], f32, tag="pp", name=f"pg{m}") for m in range(KB)
            ]
            for kb in range(2):
                for m in range(KB):
                    nc.tensor.matmul(
                        pgs[m][:],
                        lhsT=hy_sb[:, kb * _kbw + m * P : kb * _kbw + (m + 1) * P],
                        rhs=hy_sb[:, kb * _kbw + K_ATOMS : (kb + 1) * _kbw],
                        start=(kb == 0),
                        stop=(kb == 1),
                    )
            for m in range(KB):
                if niter == 1:
                    xm = opool.tile([P, BS], f32, tag="x", name=f"x1{m}")
                    nc.vector.tensor_copy(xm[:], pgs[m][:])
                    nc.sync.dma_start(outr[:, m, :], xm[:])
                else:
                    wm = wpool.tile([P, BS], f16, tag="w", name=f"w1{m}")
                    nc.scalar.activation(wm[:], pgs[m][:], Act.Abs, bias=nb[:, 0:1])
                    # Gb to SBUF with the -t*rowsum bias folded in
                    nc.scalar.activation(
                        gb_sb[:, m, :], pgs[m][:], Act.Identity,
                        bias=cn_sb[:, m : m + 1],
                    )
                    if m < 2:
                        nc.scalar.activation(
                            gb16_sb[:, m, :], pgs[m][:], Act.Identity,
                            bias=cn_sb[:, m : m + 1],
                        )
                    sm = spool.tile([P, BS], f16, tag=f"s{m}", name=f"s1{m}")
                    gbin = gb16_sb[:, m, :] if m < 2 else gb_sb[:, m, :]
                    nc.vector.scalar_tensor_tensor(
                        sm[:], pgs[m][:], THR, gbin, Alu.min, Alu.add
                    )
                    w_cur[m], s_cur[m] = wm, sm

            # ---- iterations 2..niter ----
            for it in range(2, niter + 1):
                last = it == niter
                pps = [
                    ppool.tile([P, BS], f32, tag="pp", name=f"pp{it}_{m}")
                    for m in range(KB)
                ]
                vs = [None, None]
                neww = [None] * KB
                news = [None] * KB
                for m in range(KB):
                    use_ident = (m >= 2) and not last
                    if use_ident:
                        # v' accumulates directly in PSUM: I @ S_comb + Minv @ w
                        nc.tensor.matmul(
                            pps[m][:],
                            lhsT=id_sb[:],
                            rhs=s_cur[m][:],
                            start=True,
                            stop=False,
                        )
                    for kb in range(KB):
                        nc.tensor.matmul(
                            pps[m][:],
                            lhsT=mi_sb[:, MIW + kb * K_ATOMS + m * P : MIW + kb * K_ATOMS + (m + 1) * P],
                            rhs=w_cur[kb][:],
                            start=(kb == 0) and not use_ident,
                            stop=(kb == KB - 1),
                        )
                    if last:
                        xm = opool.tile([P, BS], f32, tag="x", name=f"x{m}")
                        nc.vector.scalar_tensor_tensor(
                            xm[:], pps[m][:], 0.0, gb_sb[:, m, :], Alu.bypass, Alu.add
                        )
                        nc.sync.dma_start(outr[:, m, 0 : BS // 2], xm[:, 0 : BS // 2])
                        nc.sync.dma_start(outr[:, m, BS // 2 :], xm[:, BS // 2 :])
                    elif m < 2:
                        # V-op: v = psum + S_prev (critical chain)
                        vm = vpool.tile([P, BS], f16, tag="v", name=f"v{it}_{m}")
                        nc.vector.scalar_tensor_tensor(
                            vm[:], pps[m][:], 0.0, s_cur[m][:], Alu.bypass, Alu.add
                        )
                        vs[m] = vm
                        wm = wpool.tile([P, BS], f16, tag="w", name=f"w{it}_{m}")
                        nc.scalar.activation(wm[:], vm[:], Act.Abs, bias=nb[:, 0:1])
                        neww[m] = wm
                    else:
                        # v lives in PSUM; ACT reads it directly
                        wm = wpool.tile([P, BS], f16, tag="w", name=f"w{it}_{m}")
                        nc.scalar.activation(wm[:], pps[m][:], Act.Abs, bias=nb[:, 0:1])
                        neww[m] = wm
                        sm = spool.tile([P, BS], f16, tag=f"s{m}", name=f"s{it}_{m}")
                        nc.vector.scalar_tensor_tensor(
                            sm[:], pps[m][:], THR, gb_sb[:, m, :], Alu.min, Alu.add
                        )
                        news[m] = sm
                if not last:
                    # S ops for chunks 0,1 (off the critical chain)
                    for m in range(2):
                        sm = spool.tile([P, BS], f16, tag=f"s{m}", name=f"s{it}_{m}")
                        nc.vector.scalar_tensor_tensor(
                            sm[:], vs[m][:], THR, gb16_sb[:, m, :], Alu.min, Alu.add
                        )
                        news[m] = sm
                    w_cur, s_cur = neww, news

    nc.finalize()
    return nc


def _get_nc(niter):
    if niter not in _NC_CACHE:
        _NC_CACHE[niter] = _build(niter)
    return _NC_CACHE[niter]


def _prep_in_maps(Y, A):
    """Host precompute of the A-derived (voxel-independent) factor matrices,
    in float64: the inverse replaces the reference's Cholesky solve. Shards Y
    over voxels (transposed + augmented ones-row) and packs all device inputs
    into one pre-transposed [128, NPACK] fp16 array so every DMA descriptor
    is a multi-KB contiguous run."""
    A64 = A.astype(np.float64)
    LHS = A64.T @ A64 + RHO * np.eye(K_ATOMS)
    Minv = np.linalg.inv(LHS)
    Minv = (Minv + Minv.T) / 2
    Hm = A64 @ Minv  # [M, K]
    rsum = Minv.sum(axis=1)

    Ht = Hm.astype(np.float16)  # [M, K], M = 2*P exactly
    htp = Ht.reshape(2, P, K_ATOMS).transpose(1, 0, 2)  # [P, 2, K]
    Mi = Minv.astype(np.float16)
    mip = Mi.reshape(KB, P, K_ATOMS).transpose(1, 0, 2).reshape(P, KB * K_ATOMS)
    cneg = (-THR * rsum).astype(np.float16).reshape(KB, P).T  # [P, KB]
    Id = np.eye(P, dtype=np.float16)
    fixed = np.concatenate([Id, cneg, mip], axis=1)  # [P, P + KB + KB*K]

    in_maps = []
    for c in range(N_CORES):
        Yt = Y[c * BS : (c + 1) * BS, :].T.astype(np.float16)  # [M, BS]
        ytp = Yt.reshape(2, P, BS).transpose(1, 0, 2)  # [P, 2, BS]
        hy = np.concatenate([htp, ytp], axis=2).reshape(P, 2 * (K_ATOMS + BS))
        packed = np.ascontiguousarray(np.concatenate([hy, fixed], axis=1))
        in_maps.append({"packed": packed})
    return in_maps


def kernel(Y, A, max_iter):
    from concourse.bass_utils import run_bass_kernel_spmd

    Y = np.ascontiguousarray(np.asarray(Y, dtype=np.float32))
    A = np.ascontiguousarray(np.asarray(A, dtype=np.float32))
    niter = int(max_iter)
    assert Y.shape == (B_VOX, M_MEAS) and A.shape == (M_MEAS, K_ATOMS)
    assert niter >= 1

    in_maps = _prep_in_maps(Y, A)
    nc = _get_nc(niter)
    res = run_bass_kernel_spmd(nc, in_maps, core_ids=list(range(N_CORES)))

    outp = np.empty((B_VOX, K_ATOMS), np.float32)
    for c in range(N_CORES):
        outp[c * BS : (c + 1) * BS] = res.results[c]["out"].T
    return outp
